# revision 28
# baseline (speedup 1.0000x reference)
"""Trainium2 Bass kernel for nn_DiffKS (differentiable Karplus-Strong string).

Math:  y[t] = x[t] - sum_j vals[t,j] * y[t-1-z[t]-j],  z in [~289, ~517]
where x is the order-1-shaped excitation and vals/z come from a cubic-spline
upsampled delay/coefficient trajectory.

The feedback reaches >= ~290 samples back, so 128-sample blocks have no
intra-block dependency: 345 serial rounds of small matmuls on host-packed
dense fp16 tiles (rows = history position mod 128).

KEY HW fact (measured): LDWEIGHTS of a full 128-row fp16 tile runs at
~27ns, but ANY row-quadrant load (K<128 / tile_position row offset)
costs ~104-128ns and keeps the pipe in slow mode. So every piece here
is a FULL-K=128 matmul against one history column; rows outside a
sample-column's taps are zero in the packed tile, so full-K reads are
exact. A round whose tap band crosses a 128-position boundary is split
by OUTPUT RANGE (M, column-quadrants are fast): samples before the
crossing read col c1, samples after read c1+1, and the one straddling
32-chunk gets its cross-boundary taps via a small extra [128,32] tile
accumulated on top.

Other structure:
 - emission is dependency-readiness ordered; the in-order PE seldom
   head-of-line blocks.
 - round update h = x - acc fused (f32 PSUM -> fp16 SBUF history),
   DVE for 2 of 3 rounds, ACT for 1 of 3.
 - history striped over 8 phase tiles (col c -> tile c%8).
 - V tiles stream [128, NR, 128] contiguous, 4 groups deep; extra
   straddle tiles stream [128, NX, 32] in chunks.
Output y is recovered at the end by transposing the fp16 history phase
tiles (PE transpose) and casting to f32.
"""
import numpy as np

import concourse.bacc as bacc
import concourse.mybir as mybir
from concourse.tile import TileContext
from concourse.bass_utils import run_bass_kernel_spmd

T = 44100
NFRAMES = 100
NCOEF = 6
B = 128
NR = (T + B - 1) // B          # 345 rounds
TP = NR * B                    # 44160
OFFC = 5                       # leading zero history columns
NCOLS = NR + OFFC              # 350
GRP = 8                        # V streaming group size
F32 = mybir.dt.float32
FP16 = mybir.dt.float16
NPH = 16                       # history phase tiles
SLOTS = (NCOLS + NPH - 1) // NPH   # 44
NBANK = 6                      # rotating PSUM acc banks
ECHUNK = 64                    # extra-tile streaming chunk (crossings)


TRACE = False
LAST_EXEC_NS = None
LAST_RES = None


# ----------------------------------------------------------------- host math
def _sigmoid(v):
    return 1.0 / (1.0 + np.exp(-v))


def _spline_eval(y, n_out):
    """Natural cubic spline on uniform knots in [0,1] (float64; the f32
    reference differs by ~1e-7 relative)."""
    n, d = y.shape
    h = 1.0 / (n - 1)
    rhs = 6.0 * (y[2:] - 2.0 * y[1:-1] + y[:-2]) / h
    Tm = (np.diag(np.full(n - 2, 4.0 * h))
          + np.diag(np.full(n - 3, h), 1)
          + np.diag(np.full(n - 3, h), -1))
    M_in = np.linalg.solve(Tm, rhs)
    M = np.concatenate([np.zeros((1, d)), M_in, np.zeros((1, d))])
    t_out = np.linspace(0.0, 1.0, n_out)
    idx = np.clip((t_out / h).astype(np.int32), 0, n - 2)
    f = (t_out - idx.astype(np.float64) * h)[:, None]
    y0, y1 = y[idx], y[idx + 1]
    M0, M1 = M[idx], M[idx + 1]
    b = (y1 - y0) / h - h * (2.0 * M0 + M1) / 6.0
    c = 0.5 * M0
    dd = (M1 - M0) / (6.0 * h)
    return y0 + f * (b + f * (c + f * dd))


def _host_structure(delay_len_frames, raw_gain, raw_coeff_frames):
    gain = _sigmoid(np.float64(raw_gain))
    sig = _sigmoid(np.float64(raw_coeff_frames))
    bf = sig / sig.sum(-1, keepdims=True) * gain
    params = np.concatenate([np.float64(delay_len_frames)[:, None], bf], axis=1)
    up = _spline_eval(params, T)
    delay, b = up[:, 0], up[:, 1:]
    z = np.floor(delay).astype(np.int64)
    alfa = delay - np.floor(delay)
    first = (-(1.0 - alfa) * b[:, 0])[:, None]
    mid = -(alfa[:, None] * b[:, :-1] + (1.0 - alfa)[:, None] * b[:, 1:])
    last = (-alfa * b[:, -1])[:, None]
    vals = np.concatenate([first, mid, last], axis=1)
    vf = vals[:, ::-1].copy()          # vf[t, jj] multiplies y[t-7-z[t]+jj]
    s0 = np.arange(T) - 7 - z
    return vf, s0


def _lpc1(e, a):
    x = np.empty_like(e)
    prev = 0.0
    for t in range(len(e)):
        prev = e[t] - a[t] * prev
        x[t] = prev
    return x


# ------------------------------------------------------------ blocked plan
def _m_split(t0, t1):
    """Split output range [t0,t1) into uniform M=32 pieces: the PE pays
    ~100ns to switch tile_size, but a homogeneous (128,32) stream with
    cycling col-quadrant t0 runs at ~20ns/piece."""
    return [(a, a + 32) for a in range(t0, t1, 32)]


def _plan_group(s0p, k, a, b):
    """For chunk-group [a*32, b*32) of round k, find the best window.
    Returns (cost, info) where info = (w0r, P, tc) with P the column
    boundary position (or None) and tc the straddle chunk start (or
    None), or None if infeasible."""
    base = k * B
    t0, t1 = a * 32, b * 32
    seg = s0p[base + t0: base + t1]
    lo = int(seg.min()) + OFFC * B
    hi = int(seg.max()) + 6 + OFFC * B
    if hi - lo + 1 > B:
        return None
    wlo = -(-(hi - (B - 1)) // 32)
    whi = lo // 32
    best = None
    for wq in range(whi, wlo - 1, -1):
        w0r = wq * 32
        if lo < w0r or hi >= w0r + B:
            continue
        P = (w0r // B + 1) * B         # column boundary position
        if P > hi:                     # whole band in col c1
            npieces = len(_m_split(t0, t1))
            cost = 30 * npieces
            cand = (cost, (w0r, None, None))
        else:
            # find chunk columns; s0 is nondecreasing in t
            tc = None
            ca_end = t0                # end of colA chunks (before straddle)
            cb_start = t1
            ok = True
            for c in range(a, b):
                clo = int(s0p[base + c * 32: base + (c + 1) * 32].min()) \
                    + OFFC * B
                chi = int(s0p[base + c * 32: base + (c + 1) * 32].max()) \
                    + 6 + OFFC * B
                if chi < P:
                    ca_end = (c + 1) * 32
                elif clo >= P:
                    cb_start = min(cb_start, c * 32)
                else:
                    if tc is not None:
                        ok = False
                        break
                    tc = c * 32
            if not ok:
                continue
            # A covers colA chunks + straddle; B covers colB chunks
            a_end = (tc + 32) if tc is not None else ca_end
            b_start = cb_start
            np_ = 0
            if a_end > t0:
                np_ += len(_m_split(t0, a_end))
            if t1 > b_start:
                np_ += len(_m_split(b_start, t1))
            ns = 1 if tc is not None else 0
            cost = 30 * (np_ + ns) + 30 * ns   # extra DMA penalty
            cand = (cost, (w0r, P, tc))
        if best is None or cand[0] < best[0]:
            best = cand
    return best


def _build_plan(vf, s0):
    """plan[k] = [('v'|'e', idx, col, t0, t1)] all full-K pieces;
    vtiles (NR,128,128) float64; etiles list of (128,32) float64."""
    s0p = np.concatenate([s0, s0[-1] + 1 + np.arange(TP - T)])
    vfp = np.concatenate([vf, np.zeros((TP - T, 7))]).astype(np.float64)
    vtiles = np.zeros((NR, B, B), np.float64)
    etiles = []
    ecross_round = []
    plan = []
    INF = 10 ** 9
    NC4 = 4
    for k in range(NR):
        base = k * B
        cost = {}
        for a in range(NC4):
            for b in range(a + 1, NC4 + 1):
                r = _plan_group(s0p, k, a, b)
                if r is not None:
                    cost[(a, b)] = r
        dp = [(INF, None)] * (NC4 + 1)
        dp[0] = (0, None)
        for b in range(1, NC4 + 1):
            for a in range(b):
                if (a, b) in cost and dp[a][0] + cost[(a, b)][0] < dp[b][0]:
                    dp[b] = (dp[a][0] + cost[(a, b)][0], a)
        assert dp[NC4][0] < INF, f"round {k}: no feasible plan"
        groups = []
        b = NC4
        while b > 0:
            a = dp[b][1]
            groups.append((a, b, cost[(a, b)][1]))
            b = a
        groups.reverse()

        pieces = []
        for (a, b, (w0r, P, tc)) in groups:
            t0, t1 = a * 32, b * 32
            c1 = w0r // B
            r0 = w0r % B
            col_a = c1
            # write taps & assign pieces
            if P is None:
                # all taps in col c1
                for tt in range(t0, t1):
                    tg = base + tt
                    for jj in range(7):
                        p = int(s0p[tg]) + jj + OFFC * B
                        vtiles[k, p % B, tt] += vfp[tg, jj]
                for (m0, m1) in _m_split(t0, t1):
                    pieces.append(('v', k, col_a, m0, m1))
            else:
                a_end = (tc + 32) if tc is not None else None
                et = None
                if tc is not None:
                    et = np.zeros((B, 32), np.float64)
                # chunk col assignment identical to _plan_group
                cb_start = t1
                ca_end = t0
                for c in range(a, b):
                    clo = int(s0p[base + c * 32: base + (c + 1) * 32].min())\
                        + OFFC * B
                    chi = int(s0p[base + c * 32: base + (c + 1) * 32].max())\
                        + 6 + OFFC * B
                    if chi < P:
                        ca_end = (c + 1) * 32
                    elif clo >= P:
                        cb_start = min(cb_start, c * 32)
                for tt in range(t0, t1):
                    tg = base + tt
                    in_straddle = tc is not None and tc <= tt < tc + 32
                    for jj in range(7):
                        p = int(s0p[tg]) + jj + OFFC * B
                        if in_straddle and p >= P:
                            et[p % B, tt - tc] += vfp[tg, jj]
                        else:
                            vtiles[k, p % B, tt] += vfp[tg, jj]
                a_end2 = (tc + 32) if tc is not None else ca_end
                if a_end2 > t0:
                    for (m0, m1) in _m_split(t0, a_end2):
                        pieces.append(('v', k, col_a, m0, m1))
                if t1 > cb_start:
                    for (m0, m1) in _m_split(cb_start, t1):
                        pieces.append(('v', k, col_a + 1, m0, m1))
                if tc is not None:
                    ei = len(etiles)
                    etiles.append(et)
                    ecross_round.append(k)
                    pieces.append(('e', ei, col_a + 1, tc, tc + 32))
        plan.append(pieces)
    return plan, vtiles, etiles, ecross_round


def _group_bounds():
    gb = [0, 2, 4, 8]
    while gb[-1] < NR:
        gb.append(min(gb[-1] + GRP, NR))
    g_of = np.zeros(NR, np.int64)
    for g in range(len(gb) - 1):
        g_of[gb[g]:gb[g + 1]] = g
    return gb, g_of


def _schedule(plan):
    """sched[k] = [(src, idx, col, t0, t1, start, stop)] emitted at
    iteration k, readiness-ordered (piece reading col c is ready after
    sub(c-OFFC); emit at e = clamp(d+2, group_start, j)); within an
    iteration older deps first and round-k pieces before future ones.
    start/stop per (round, t-range) region in emission order."""
    gb, g_of = _group_bounds()
    items = []
    for j, pieces in enumerate(plan):
        for (src, idx, col, t0, t1) in pieces:
            d = col - OFFC
            e = max(d + 2, gb[int(g_of[j])])
            e = min(e, j)
            items.append((e, d, j != e, j, src == 'e',
                          (src, idx, col, t0, t1)))
    items.sort(key=lambda it: (it[0], it[1], it[2], it[3], it[4]))
    first_of, last_of = {}, {}
    for i, (e, d, fut, j, isex, p) in enumerate(items):
        # region key: overlapping ranges share a region via t0 anchor:
        # 'e' pieces overlap the colA piece containing their range, so
        # anchor regions by (j, covering piece span). Use (j, t0, t1) and
        # mark 'e' pieces as accumulating into their covering region.
        tr = (j, p[3], p[4])
        if tr not in first_of:
            first_of[tr] = i
        last_of[tr] = i
    sched = [[] for _ in range(NR)]
    for i, (e, d, fut, j, isex, p) in enumerate(items):
        (src, idx, col, t0, t1) = p
        if src == 'e':
            st, sp = False, True
        else:
            tr = (j, t0, t1)
            st, sp = first_of[tr] == i, last_of[tr] == i
        sched[e].append((src, idx, j, col, t0, t1, st, sp))
    return sched


# ------------------------------------------------------------- device build
def _build_kernel(plan, n_etiles, ecross_round):
    sched = _schedule(plan)
    gb, g_of = _group_bounds()
    NX = max(1, n_etiles)
    NEC = (NX + ECHUNK - 1) // ECHUNK

    nc = bacc.Bacc("TRN2", target_bir_lowering=False, debug=False)
    v_d = nc.dram_tensor("vtiles", [B, NR, B], FP16, kind="ExternalInput")
    e_d = nc.dram_tensor("etiles", [B, NEC * ECHUNK, 32], FP16,
                         kind="ExternalInput")
    x_d = nc.dram_tensor("xcols", [B, NR], F32, kind="ExternalInput")
    id_d = nc.dram_tensor("ident", [B, B], FP16, kind="ExternalInput")
    y_d = nc.dram_tensor("y", [SLOTS * NPH * B], F32, kind="ExternalOutput")

    ident_act = mybir.ActivationFunctionType.Identity
    act_copy = mybir.ActivationFunctionType.Copy

    # iteration at which to issue each extra chunk DMA
    echunk_issue = {}
    for c in range(NEC):
        i0 = c * ECHUNK
        rnd = ecross_round[i0] if i0 < len(ecross_round) else 0
        echunk_issue.setdefault(max(0, rnd - 30), []).append(c)

    with TileContext(nc) as tc:
        with (
            tc.tile_pool(name="vpool", bufs=4) as vpool,
            tc.tile_pool(name="epool", bufs=1) as epool,
            tc.tile_pool(name="hpool", bufs=1) as hpool,
            tc.tile_pool(name="xpool", bufs=1) as xpool,
            tc.tile_pool(name="ps", bufs=NBANK, space="PSUM") as ps,
            tc.tile_pool(name="pso", bufs=2, space="PSUM") as pso,
            tc.tile_pool(name="opool", bufs=2) as opool,
        ):
            h_ph = []
            for i in range(NPH):
                ht = hpool.tile([B, SLOTS], FP16, tag=f"h{i}", name=f"h{i}")
                nc.vector.memset(ht[:, :], 0.0)
                h_ph.append(ht)
            xt = xpool.tile([B, NR], F32)
            nc.sync.dma_start(xt[:, :], x_d[:, :])
            idt = xpool.tile([B, B], FP16, tag="ident")
            nc.sync.dma_start(idt[:, :], id_d[:, :])
            et_all = epool.tile([B, NEC * ECHUNK, 32], FP16, tag="ex")

            vtiles_sb = {}
            accs = {}
            for k in range(NR):
                g = int(g_of[k])
                if k == gb[g]:
                    gn = gb[g + 1] - gb[g]
                    vt = vpool.tile([B, GRP, B], FP16, tag="v", name=f"v{g}")
                    eng = nc.sync if (g % 2 == 0) else nc.gpsimd
                    eng.dma_start(vt[:, 0:gn, :], v_d[:, gb[g]:gb[g + 1], :])
                    vtiles_sb[g] = vt
                for c in echunk_issue.get(k, []):
                    nc.gpsimd.dma_start(
                        et_all[:, c * ECHUNK:(c + 1) * ECHUNK, :],
                        e_d[:, c * ECHUNK:(c + 1) * ECHUNK, :])
                for (src, idx, j, col, t0, t1, st, sp) in sched[k]:
                    if j not in accs:
                        accs[j] = ps.tile([B, 1], F32, tag="acc",
                                          name=f"acc{j}")
                    hcol = h_ph[col % NPH][:, col // NPH:col // NPH + 1]
                    if src == 'v':
                        vt = vtiles_sb[int(g_of[j])]
                        kk = j - gb[int(g_of[j])]
                        lhsT = vt[:, kk, t0:t1]
                    else:
                        lhsT = et_all[:, idx, 0:t1 - t0]
                    nc.tensor.matmul(
                        accs[j][t0:t1, :], lhsT, hcol,
                        start=st, stop=sp, tile_position=(0, t0),
                        skip_group_check=True)
                # h_col = x - acc (fused, fp16 out); DVE 2/3, ACT 1/3
                dst = k + OFFC
                hdst = h_ph[dst % NPH][:, dst // NPH:dst // NPH + 1]
                acc = accs.pop(k)
                if k % 3 != 2:
                    nc.vector.tensor_sub(hdst, xt[:, k:k + 1], acc[:, :])
                else:
                    nc.scalar.activation(hdst, acc[:, :], ident_act,
                                         bias=xt[:, k:k + 1], scale=-1.0)

            # ---- output: per-phase transpose of history -> linear y
            y3 = y_d.rearrange("(s q p) -> s q p", q=NPH, p=B)
            for i in range(NPH):
                tp = pso.tile([SLOTS, B], FP16, tag="tp", name=f"tp{i}")
                nc.tensor.transpose(tp[:, :], h_ph[i][:, :], idt[:, :])
                osb = opool.tile([SLOTS, B], F32, tag="o", name=f"o{i}")
                if i % 2 == 0:
                    nc.vector.tensor_copy(osb[:, :], tp[:, :])
                else:
                    nc.scalar.activation(osb[:, :], tp[:, :], act_copy)
                if i >= OFFC:
                    nc.sync.dma_start(y3[0:SLOTS, i - OFFC, :],
                                      osb[0:SLOTS, :])
                else:
                    nc.sync.dma_start(y3[0:SLOTS - 1, i + NPH - OFFC, :],
                                      osb[1:SLOTS, :])
    nc.compile()
    return nc


# --------------------------------------------------------------- entry point
_CACHE = {}


def kernel(delay_len_frames, raw_gain, raw_coeff_frames, excitation,
           exc_coefficients, n_samples):
    delay_len_frames = np.asarray(delay_len_frames, np.float32)
    raw_gain = np.asarray(raw_gain, np.float32)
    raw_coeff_frames = np.asarray(raw_coeff_frames, np.float32)
    excitation = np.asarray(excitation, np.float32)
    exc_coefficients = np.asarray(exc_coefficients, np.float32)
    assert int(n_samples) == T

    vf, s0 = _host_structure(delay_len_frames, raw_gain[0], raw_coeff_frames)
    plan, vtiles, etiles, ecross_round = _build_plan(vf, s0)

    vpack = np.ascontiguousarray(
        vtiles.astype(np.float16).transpose(1, 0, 2))
    NX = max(1, len(etiles))
    NEC = (NX + ECHUNK - 1) // ECHUNK
    epack = np.zeros((B, NEC * ECHUNK, 32), np.float16)
    for i, et in enumerate(etiles):
        epack[:, i, :] = et.astype(np.float16)

    x = _lpc1(np.float64(excitation), np.float64(exc_coefficients[0, :, 0]))
    xp = np.zeros(TP, np.float32)
    xp[:T] = x.astype(np.float32)
    xcols = np.ascontiguousarray(xp.reshape(NR, B).T)   # [128, NR]

    key = hash((delay_len_frames.tobytes(), raw_gain.tobytes(),
                raw_coeff_frames.tobytes()))
    if key not in _CACHE:
        _CACHE[key] = _build_kernel(plan, len(etiles), ecross_round)
    nc = _CACHE[key]

    in_map = dict(vtiles=vpack, etiles=epack, xcols=xcols,
                  ident=np.eye(B, dtype=np.float16))
    res = run_bass_kernel_spmd(nc, [in_map], core_ids=[0], trace=TRACE)
    if TRACE:
        global LAST_EXEC_NS, LAST_RES
        LAST_EXEC_NS = res.exec_time_ns
        LAST_RES = res
    y = res.results[0]["y"]
    return np.asarray(y[:T], np.float32)


if __name__ == "__main__":
    rng = np.random.default_rng(0)
    out = kernel(
        delay_len_frames=300 + 200 * rng.random(NFRAMES).astype(np.float32),
        raw_gain=np.full(1, 2.5, np.float32),
        raw_coeff_frames=(-2 * rng.random((NFRAMES, NCOEF))).astype(np.float32),
        excitation=rng.standard_normal(T).astype(np.float32),
        exc_coefficients=0.01 * rng.standard_normal((1, T, 1)).astype(np.float32),
        n_samples=T)
    print("kernel ran, out:", out.shape, out[:4])


# revision 36
# speedup vs baseline: 1.0057x; 1.0057x over previous
"""Trainium2 Bass kernel for nn_DiffKS (differentiable Karplus-Strong string).

Math:  y[t] = x[t] - sum_j vals[t,j] * y[t-1-z[t]-j],  z in [~289, ~517]
where x is the order-1-shaped excitation and vals/z come from a cubic-spline
upsampled delay/coefficient trajectory.

The feedback reaches >= ~290 samples back, so 128-sample blocks have no
intra-block dependency: 345 serial rounds of small matmuls on host-packed
dense fp16 tiles (rows = history position mod 128).

KEY HW fact (measured): LDWEIGHTS of a full 128-row fp16 tile runs at
~27ns, but ANY row-quadrant load (K<128 / tile_position row offset)
costs ~104-128ns and keeps the pipe in slow mode. So every piece here
is a FULL-K=128 matmul against one history column; rows outside a
sample-column's taps are zero in the packed tile, so full-K reads are
exact. A round whose tap band crosses a 128-position boundary is split
by OUTPUT RANGE (M, column-quadrants are fast): samples before the
crossing read col c1, samples after read c1+1, and the one straddling
32-chunk gets its cross-boundary taps via a small extra [128,32] tile
accumulated on top.

Other structure:
 - emission is dependency-readiness ordered; the in-order PE seldom
   head-of-line blocks.
 - round update h = x - acc fused (f32 PSUM -> fp16 SBUF history),
   DVE for 2 of 3 rounds, ACT for 1 of 3.
 - history striped over 8 phase tiles (col c -> tile c%8).
 - V tiles stream [128, NR, 128] contiguous, 4 groups deep; extra
   straddle tiles stream [128, NX, 32] in chunks.
Output y is recovered at the end by transposing the fp16 history phase
tiles (PE transpose) and casting to f32.
"""
import numpy as np

import concourse.bacc as bacc
import concourse.mybir as mybir
from concourse.tile import TileContext
from concourse.bass_utils import run_bass_kernel_spmd

T = 44100
NFRAMES = 100
NCOEF = 6
B = 128
NR = (T + B - 1) // B          # 345 rounds
TP = NR * B                    # 44160
OFFC = 5                       # leading zero history columns
NCOLS = NR + OFFC              # 350
GRP = 8                        # V streaming group size
F32 = mybir.dt.float32
FP16 = mybir.dt.float16
NPH = 8                        # history phase tiles
SLOTS = (NCOLS + NPH - 1) // NPH   # 44
NBANK = 6                      # rotating PSUM acc banks
ECHUNK = 64                    # extra-tile streaming chunk (crossings)


TRACE = False
LAST_EXEC_NS = None
LAST_RES = None


# ----------------------------------------------------------------- host math
def _sigmoid(v):
    return 1.0 / (1.0 + np.exp(-v))


def _spline_eval(y, n_out):
    """Natural cubic spline on uniform knots in [0,1] (float64; the f32
    reference differs by ~1e-7 relative)."""
    n, d = y.shape
    h = 1.0 / (n - 1)
    rhs = 6.0 * (y[2:] - 2.0 * y[1:-1] + y[:-2]) / h
    Tm = (np.diag(np.full(n - 2, 4.0 * h))
          + np.diag(np.full(n - 3, h), 1)
          + np.diag(np.full(n - 3, h), -1))
    M_in = np.linalg.solve(Tm, rhs)
    M = np.concatenate([np.zeros((1, d)), M_in, np.zeros((1, d))])
    t_out = np.linspace(0.0, 1.0, n_out)
    idx = np.clip((t_out / h).astype(np.int32), 0, n - 2)
    f = (t_out - idx.astype(np.float64) * h)[:, None]
    y0, y1 = y[idx], y[idx + 1]
    M0, M1 = M[idx], M[idx + 1]
    b = (y1 - y0) / h - h * (2.0 * M0 + M1) / 6.0
    c = 0.5 * M0
    dd = (M1 - M0) / (6.0 * h)
    return y0 + f * (b + f * (c + f * dd))


def _host_structure(delay_len_frames, raw_gain, raw_coeff_frames):
    gain = _sigmoid(np.float64(raw_gain))
    sig = _sigmoid(np.float64(raw_coeff_frames))
    bf = sig / sig.sum(-1, keepdims=True) * gain
    params = np.concatenate([np.float64(delay_len_frames)[:, None], bf], axis=1)
    up = _spline_eval(params, T)
    delay, b = up[:, 0], up[:, 1:]
    z = np.floor(delay).astype(np.int64)
    alfa = delay - np.floor(delay)
    first = (-(1.0 - alfa) * b[:, 0])[:, None]
    mid = -(alfa[:, None] * b[:, :-1] + (1.0 - alfa)[:, None] * b[:, 1:])
    last = (-alfa * b[:, -1])[:, None]
    vals = np.concatenate([first, mid, last], axis=1)
    vf = vals[:, ::-1].copy()          # vf[t, jj] multiplies y[t-7-z[t]+jj]
    s0 = np.arange(T) - 7 - z
    return vf, s0


def _lpc1(e, a):
    x = np.empty_like(e)
    prev = 0.0
    for t in range(len(e)):
        prev = e[t] - a[t] * prev
        x[t] = prev
    return x


# ------------------------------------------------------------ blocked plan
def _m_split(t0, t1):
    """Split output range [t0,t1) into uniform M=32 pieces: the PE pays
    ~100ns to switch tile_size, but a homogeneous (128,32) stream with
    cycling col-quadrant t0 runs at ~20ns/piece."""
    return [(a, a + 32) for a in range(t0, t1, 32)]


def _plan_group(s0p, k, a, b):
    """For chunk-group [a*32, b*32) of round k, find the best window.
    Returns (cost, info) where info = (w0r, P, tc) with P the column
    boundary position (or None) and tc the straddle chunk start (or
    None), or None if infeasible."""
    base = k * B
    t0, t1 = a * 32, b * 32
    seg = s0p[base + t0: base + t1]
    lo = int(seg.min()) + OFFC * B
    hi = int(seg.max()) + 6 + OFFC * B
    if hi - lo + 1 > B:
        return None
    wlo = -(-(hi - (B - 1)) // 32)
    whi = lo // 32
    best = None
    for wq in range(whi, wlo - 1, -1):
        w0r = wq * 32
        if lo < w0r or hi >= w0r + B:
            continue
        P = (w0r // B + 1) * B         # column boundary position
        if P > hi:                     # whole band in col c1
            npieces = len(_m_split(t0, t1))
            cost = 30 * npieces
            cand = (cost, (w0r, None, None))
        else:
            # find chunk columns; s0 is nondecreasing in t
            tc = None
            ca_end = t0                # end of colA chunks (before straddle)
            cb_start = t1
            ok = True
            for c in range(a, b):
                clo = int(s0p[base + c * 32: base + (c + 1) * 32].min()) \
                    + OFFC * B
                chi = int(s0p[base + c * 32: base + (c + 1) * 32].max()) \
                    + 6 + OFFC * B
                if chi < P:
                    ca_end = (c + 1) * 32
                elif clo >= P:
                    cb_start = min(cb_start, c * 32)
                else:
                    if tc is not None:
                        ok = False
                        break
                    tc = c * 32
            if not ok:
                continue
            # A covers colA chunks + straddle; B covers colB chunks
            a_end = (tc + 32) if tc is not None else ca_end
            b_start = cb_start
            np_ = 0
            if a_end > t0:
                np_ += len(_m_split(t0, a_end))
            if t1 > b_start:
                np_ += len(_m_split(b_start, t1))
            ns = 1 if tc is not None else 0
            cost = 30 * (np_ + ns) + 30 * ns   # extra DMA penalty
            cand = (cost, (w0r, P, tc))
        if best is None or cand[0] < best[0]:
            best = cand
    return best


def _build_plan(vf, s0):
    """plan[k] = [('v'|'e', idx, col, t0, t1)] all full-K pieces;
    vtiles (NR,128,128) float64; etiles list of (128,32) float64."""
    s0p = np.concatenate([s0, s0[-1] + 1 + np.arange(TP - T)])
    vfp = np.concatenate([vf, np.zeros((TP - T, 7))]).astype(np.float64)
    vtiles = np.zeros((NR, B, B), np.float64)
    etiles = []
    ecross_round = []
    plan = []
    INF = 10 ** 9
    NC4 = 4
    for k in range(NR):
        base = k * B
        cost = {}
        for a in range(NC4):
            for b in range(a + 1, NC4 + 1):
                r = _plan_group(s0p, k, a, b)
                if r is not None:
                    cost[(a, b)] = r
        dp = [(INF, None)] * (NC4 + 1)
        dp[0] = (0, None)
        for b in range(1, NC4 + 1):
            for a in range(b):
                if (a, b) in cost and dp[a][0] + cost[(a, b)][0] < dp[b][0]:
                    dp[b] = (dp[a][0] + cost[(a, b)][0], a)
        assert dp[NC4][0] < INF, f"round {k}: no feasible plan"
        groups = []
        b = NC4
        while b > 0:
            a = dp[b][1]
            groups.append((a, b, cost[(a, b)][1]))
            b = a
        groups.reverse()

        pieces = []
        for (a, b, (w0r, P, tc)) in groups:
            t0, t1 = a * 32, b * 32
            c1 = w0r // B
            r0 = w0r % B
            col_a = c1
            # write taps & assign pieces
            if P is None:
                # all taps in col c1
                for tt in range(t0, t1):
                    tg = base + tt
                    for jj in range(7):
                        p = int(s0p[tg]) + jj + OFFC * B
                        vtiles[k, p % B, tt] += vfp[tg, jj]
                for (m0, m1) in _m_split(t0, t1):
                    pieces.append(('v', k, col_a, m0, m1))
            else:
                a_end = (tc + 32) if tc is not None else None
                et = None
                if tc is not None:
                    et = np.zeros((B, 32), np.float64)
                # chunk col assignment identical to _plan_group
                cb_start = t1
                ca_end = t0
                for c in range(a, b):
                    clo = int(s0p[base + c * 32: base + (c + 1) * 32].min())\
                        + OFFC * B
                    chi = int(s0p[base + c * 32: base + (c + 1) * 32].max())\
                        + 6 + OFFC * B
                    if chi < P:
                        ca_end = (c + 1) * 32
                    elif clo >= P:
                        cb_start = min(cb_start, c * 32)
                for tt in range(t0, t1):
                    tg = base + tt
                    in_straddle = tc is not None and tc <= tt < tc + 32
                    for jj in range(7):
                        p = int(s0p[tg]) + jj + OFFC * B
                        if in_straddle and p >= P:
                            et[p % B, tt - tc] += vfp[tg, jj]
                        else:
                            vtiles[k, p % B, tt] += vfp[tg, jj]
                a_end2 = (tc + 32) if tc is not None else ca_end
                if a_end2 > t0:
                    for (m0, m1) in _m_split(t0, a_end2):
                        pieces.append(('v', k, col_a, m0, m1))
                if t1 > cb_start:
                    for (m0, m1) in _m_split(cb_start, t1):
                        pieces.append(('v', k, col_a + 1, m0, m1))
                if tc is not None:
                    ei = len(etiles)
                    etiles.append(et)
                    ecross_round.append(k)
                    pieces.append(('e', ei, col_a + 1, tc, tc + 32))
        plan.append(pieces)
    return plan, vtiles, etiles, ecross_round


def _group_bounds():
    gb = [0, 2, 4, 8]
    while gb[-1] < NR:
        gb.append(min(gb[-1] + GRP, NR))
    g_of = np.zeros(NR, np.int64)
    for g in range(len(gb) - 1):
        g_of[gb[g]:gb[g + 1]] = g
    return gb, g_of


def _sub_iter(m):
    """Iteration at which pair m's sub (rounds 2m, 2m+1) is emitted."""
    return min(2 * m + 1, NR - 1)


def _schedule(plan):
    """sched[k] = [(src, idx, col, t0, t1, start, stop)] emitted at
    iteration k, readiness-ordered: piece reading col c is ready after
    the PAIR sub of pair (c-OFFC)//2; emit at e = clamp(prod+1,
    group_start, j). start/stop per (round, t-range) region in emission
    order."""
    gb, g_of = _group_bounds()
    items = []
    for j, pieces in enumerate(plan):
        for (src, idx, col, t0, t1) in pieces:
            d = col - OFFC
            prod = _sub_iter(d // 2) if d >= 0 else -1
            e = max(prod + 1, gb[int(g_of[j])])
            e = min(e, j)
            items.append((e, prod, j != e, j, src == 'e',
                          (src, idx, col, t0, t1)))
    items.sort(key=lambda it: (it[0], it[1], it[2], it[3], it[4]))
    first_of, last_of = {}, {}
    for i, (e, d, fut, j, isex, p) in enumerate(items):
        # region key: overlapping ranges share a region via t0 anchor:
        # 'e' pieces overlap the colA piece containing their range, so
        # anchor regions by (j, covering piece span). Use (j, t0, t1) and
        # mark 'e' pieces as accumulating into their covering region.
        tr = (j, p[3], p[4])
        if tr not in first_of:
            first_of[tr] = i
        last_of[tr] = i
    sched = [[] for _ in range(NR)]
    for i, (e, d, fut, j, isex, p) in enumerate(items):
        (src, idx, col, t0, t1) = p
        if src == 'e':
            st, sp = False, True
        else:
            tr = (j, t0, t1)
            st, sp = first_of[tr] == i, last_of[tr] == i
        sched[e].append((src, idx, j, col, t0, t1, st, sp))
    return sched


# ------------------------------------------------------------- device build
def _build_kernel(plan, n_etiles, ecross_round):
    sched = _schedule(plan)
    gb, g_of = _group_bounds()
    NX = max(1, n_etiles)
    NEC = (NX + ECHUNK - 1) // ECHUNK

    nc = bacc.Bacc("TRN2", target_bir_lowering=False, debug=False)
    v_d = nc.dram_tensor("vtiles", [B, NR, B], FP16, kind="ExternalInput")
    e_d = nc.dram_tensor("etiles", [B, NEC * ECHUNK, 32], FP16,
                         kind="ExternalInput")
    x_d = nc.dram_tensor("xcols", [B, NR + 1], F32, kind="ExternalInput")
    id_d = nc.dram_tensor("ident", [B, B], FP16, kind="ExternalInput")
    y_d = nc.dram_tensor("y", [SLOTS * NPH * B], F32, kind="ExternalOutput")

    ident_act = mybir.ActivationFunctionType.Identity
    act_copy = mybir.ActivationFunctionType.Copy

    # iteration at which to issue each extra chunk DMA
    echunk_issue = {}
    for c in range(NEC):
        i0 = c * ECHUNK
        rnd = ecross_round[i0] if i0 < len(ecross_round) else 0
        echunk_issue.setdefault(max(0, rnd - 30), []).append(c)

    with TileContext(nc) as tc:
        with (
            tc.tile_pool(name="vpool", bufs=4) as vpool,
            tc.tile_pool(name="epool", bufs=1) as epool,
            tc.tile_pool(name="hpool", bufs=1) as hpool,
            tc.tile_pool(name="xpool", bufs=1) as xpool,
            tc.tile_pool(name="ps", bufs=NBANK, space="PSUM") as ps,
            tc.tile_pool(name="pso", bufs=2, space="PSUM") as pso,
            tc.tile_pool(name="opool", bufs=2) as opool,
        ):
            h_ph = []
            for i in range(NPH):
                ht = hpool.tile([B, SLOTS], FP16, tag=f"h{i}", name=f"h{i}")
                nc.vector.memset(ht[:, :], 0.0)
                h_ph.append(ht)
            hz = hpool.tile([B, OFFC], FP16, tag="hz", name="hz")
            nc.vector.memset(hz[:, :], 0.0)

            def hcol_ap(c):
                if c < OFFC:
                    return hz[:, c:c + 1]
                cp, par = (c - OFFC) // 2, (c - OFFC) % 2
                s = 2 * (cp // NPH) + par
                return h_ph[cp % NPH][:, s:s + 1]

            xt = xpool.tile([B, NR + 1], F32)
            nc.sync.dma_start(xt[:, :], x_d[:, :])
            idt = xpool.tile([B, B], FP16, tag="ident")
            nc.sync.dma_start(idt[:, :], id_d[:, :])
            et_all = epool.tile([B, NEC * ECHUNK, 32], FP16, tag="ex")

            vtiles_sb = {}
            accs = {}
            for k in range(NR):
                g = int(g_of[k])
                if k == gb[g]:
                    gn = gb[g + 1] - gb[g]
                    vt = vpool.tile([B, GRP, B], FP16, tag="v", name=f"v{g}")
                    eng = nc.sync if (g % 2 == 0) else nc.gpsimd
                    eng.dma_start(vt[:, 0:gn, :], v_d[:, gb[g]:gb[g + 1], :])
                    vtiles_sb[g] = vt
                for c in echunk_issue.get(k, []):
                    nc.gpsimd.dma_start(
                        et_all[:, c * ECHUNK:(c + 1) * ECHUNK, :],
                        e_d[:, c * ECHUNK:(c + 1) * ECHUNK, :])
                for (src, idx, j, col, t0, t1, st, sp) in sched[k]:
                    m = j // 2
                    if m not in accs:
                        accs[m] = ps.tile([B, 2], F32, tag="acc",
                                          name=f"acc{m}")
                    if src == 'v':
                        vt = vtiles_sb[int(g_of[j])]
                        kk = j - gb[int(g_of[j])]
                        lhsT = vt[:, kk, t0:t1]
                    else:
                        lhsT = et_all[:, idx, 0:t1 - t0]
                    nc.tensor.matmul(
                        accs[m][t0:t1, j % 2:j % 2 + 1], lhsT, hcol_ap(col),
                        start=st, stop=sp, tile_position=(0, t0),
                        skip_group_check=True)
                # pair sub: h cols (2m+OFFC, 2m+OFFC+1) = x - acc, on DVE
                if k % 2 == 1 or k == NR - 1:
                    m = k // 2
                    s = 2 * (m // NPH)
                    hdst = h_ph[m % NPH][:, s:s + 2]
                    acc = accs.pop(m)
                    nc.vector.tensor_sub(hdst, xt[:, 2 * m:2 * m + 2],
                                         acc[:, :])

            # ---- output: per-phase transpose of history -> linear y
            # phase i slot 2q+par = pair cp = 8q+i, round k = 2cp+par =
            # 16q + 2i + par; y viewed [q, w=16, p]: w = 2i + par.
            y4 = y_d.rearrange("(q w p) -> q w p", w=2 * NPH, p=B)
            S2 = SLOTS // 2
            for i in range(NPH):
                for par in (0, 1):
                    tp = pso.tile([S2, B], FP16, tag="tp", name=f"tp{i}{par}")
                    nc.tensor.transpose(tp[:, :], h_ph[i][:, par:SLOTS:2],
                                        idt[:, :])
                    osb = opool.tile([S2, B], F32, tag="o", name=f"o{i}{par}")
                    if par == 0:
                        nc.vector.tensor_copy(osb[:, :], tp[:, :])
                    else:
                        nc.scalar.activation(osb[:, :], tp[:, :], act_copy)
                    nc.sync.dma_start(y4[0:S2, 2 * i + par, :], osb[:, :])
    nc.compile()
    return nc


# --------------------------------------------------------------- entry point
_CACHE = {}


def kernel(delay_len_frames, raw_gain, raw_coeff_frames, excitation,
           exc_coefficients, n_samples):
    delay_len_frames = np.asarray(delay_len_frames, np.float32)
    raw_gain = np.asarray(raw_gain, np.float32)
    raw_coeff_frames = np.asarray(raw_coeff_frames, np.float32)
    excitation = np.asarray(excitation, np.float32)
    exc_coefficients = np.asarray(exc_coefficients, np.float32)
    assert int(n_samples) == T

    vf, s0 = _host_structure(delay_len_frames, raw_gain[0], raw_coeff_frames)
    plan, vtiles, etiles, ecross_round = _build_plan(vf, s0)

    vpack = np.ascontiguousarray(
        vtiles.astype(np.float16).transpose(1, 0, 2))
    NX = max(1, len(etiles))
    NEC = (NX + ECHUNK - 1) // ECHUNK
    epack = np.zeros((B, NEC * ECHUNK, 32), np.float16)
    for i, et in enumerate(etiles):
        epack[:, i, :] = et.astype(np.float16)

    x = _lpc1(np.float64(excitation), np.float64(exc_coefficients[0, :, 0]))
    xp = np.zeros((NR + 1) * B, np.float32)
    xp[:T] = x.astype(np.float32)
    xcols = np.ascontiguousarray(xp.reshape(NR + 1, B).T)   # [128, NR+1]

    key = hash((delay_len_frames.tobytes(), raw_gain.tobytes(),
                raw_coeff_frames.tobytes()))
    if key not in _CACHE:
        _CACHE[key] = _build_kernel(plan, len(etiles), ecross_round)
    nc = _CACHE[key]

    in_map = dict(vtiles=vpack, etiles=epack, xcols=xcols,
                  ident=np.eye(B, dtype=np.float16))
    res = run_bass_kernel_spmd(nc, [in_map], core_ids=[0], trace=TRACE)
    if TRACE:
        global LAST_EXEC_NS, LAST_RES
        LAST_EXEC_NS = res.exec_time_ns
        LAST_RES = res
    y = res.results[0]["y"]
    return np.asarray(y[:T], np.float32)


if __name__ == "__main__":
    rng = np.random.default_rng(0)
    out = kernel(
        delay_len_frames=300 + 200 * rng.random(NFRAMES).astype(np.float32),
        raw_gain=np.full(1, 2.5, np.float32),
        raw_coeff_frames=(-2 * rng.random((NFRAMES, NCOEF))).astype(np.float32),
        excitation=rng.standard_normal(T).astype(np.float32),
        exc_coefficients=0.01 * rng.standard_normal((1, T, 1)).astype(np.float32),
        n_samples=T)
    print("kernel ran, out:", out.shape, out[:4])


# revision 38
# speedup vs baseline: 1.1144x; 1.1080x over previous
"""Trainium2 Bass kernel for nn_DiffKS (differentiable Karplus-Strong string).

Math:  y[t] = x[t] - sum_j vals[t,j] * y[t-1-z[t]-j],  z in [~289, ~517]
where x is the order-1-shaped excitation and vals/z come from a cubic-spline
upsampled delay/coefficient trajectory.

The feedback reaches >= ~290 samples back, so 128-sample blocks have no
intra-block dependency: 345 serial rounds of small matmuls on host-packed
dense fp16 tiles (rows = history position mod 128).

KEY HW fact (measured): LDWEIGHTS of a full 128-row fp16 tile runs at
~27ns, but ANY row-quadrant load (K<128 / tile_position row offset)
costs ~104-128ns and keeps the pipe in slow mode. So every piece here
is a FULL-K=128 matmul against one history column; rows outside a
sample-column's taps are zero in the packed tile, so full-K reads are
exact. A round whose tap band crosses a 128-position boundary is split
by OUTPUT RANGE (M, column-quadrants are fast): samples before the
crossing read col c1, samples after read c1+1, and the one straddling
32-chunk gets its cross-boundary taps via a small extra [128,32] tile
accumulated on top.

Other structure:
 - emission is dependency-readiness ordered; the in-order PE seldom
   head-of-line blocks.
 - round update h = x - acc fused (f32 PSUM -> fp16 SBUF history),
   DVE for 2 of 3 rounds, ACT for 1 of 3.
 - history striped over 8 phase tiles (col c -> tile c%8).
 - V tiles stream [128, NR, 128] contiguous, 4 groups deep; extra
   straddle tiles stream [128, NX, 32] in chunks.
Output y is recovered at the end by transposing the fp16 history phase
tiles (PE transpose) and casting to f32.
"""
import numpy as np

import concourse.bacc as bacc
import concourse.mybir as mybir
from concourse.tile import TileContext
from concourse.bass_utils import run_bass_kernel_spmd

T = 44100
NFRAMES = 100
NCOEF = 6
B = 128
NR = (T + B - 1) // B          # 345 rounds
TP = NR * B                    # 44160
OFFC = 5                       # leading zero history columns
NCOLS = NR + OFFC              # 350
GRP = 8                        # V streaming group size
F32 = mybir.dt.float32
FP16 = mybir.dt.float16
NPH = 8                        # history phase tiles
SLOTS = (NCOLS + NPH - 1) // NPH   # 44
NBANK = 6                      # rotating PSUM acc banks
ECHUNK = 64                    # extra-tile streaming chunk (crossings)


TRACE = False
LAST_EXEC_NS = None
LAST_RES = None


# ----------------------------------------------------------------- host math
def _sigmoid(v):
    return 1.0 / (1.0 + np.exp(-v))


def _spline_eval(y, n_out):
    """Natural cubic spline on uniform knots in [0,1] (float64; the f32
    reference differs by ~1e-7 relative)."""
    n, d = y.shape
    h = 1.0 / (n - 1)
    rhs = 6.0 * (y[2:] - 2.0 * y[1:-1] + y[:-2]) / h
    Tm = (np.diag(np.full(n - 2, 4.0 * h))
          + np.diag(np.full(n - 3, h), 1)
          + np.diag(np.full(n - 3, h), -1))
    M_in = np.linalg.solve(Tm, rhs)
    M = np.concatenate([np.zeros((1, d)), M_in, np.zeros((1, d))])
    t_out = np.linspace(0.0, 1.0, n_out)
    idx = np.clip((t_out / h).astype(np.int32), 0, n - 2)
    f = (t_out - idx.astype(np.float64) * h)[:, None]
    y0, y1 = y[idx], y[idx + 1]
    M0, M1 = M[idx], M[idx + 1]
    b = (y1 - y0) / h - h * (2.0 * M0 + M1) / 6.0
    c = 0.5 * M0
    dd = (M1 - M0) / (6.0 * h)
    return y0 + f * (b + f * (c + f * dd))


def _host_structure(delay_len_frames, raw_gain, raw_coeff_frames):
    gain = _sigmoid(np.float64(raw_gain))
    sig = _sigmoid(np.float64(raw_coeff_frames))
    bf = sig / sig.sum(-1, keepdims=True) * gain
    params = np.concatenate([np.float64(delay_len_frames)[:, None], bf], axis=1)
    up = _spline_eval(params, T)
    delay, b = up[:, 0], up[:, 1:]
    z = np.floor(delay).astype(np.int64)
    alfa = delay - np.floor(delay)
    first = (-(1.0 - alfa) * b[:, 0])[:, None]
    mid = -(alfa[:, None] * b[:, :-1] + (1.0 - alfa)[:, None] * b[:, 1:])
    last = (-alfa * b[:, -1])[:, None]
    vals = np.concatenate([first, mid, last], axis=1)
    vf = vals[:, ::-1].copy()          # vf[t, jj] multiplies y[t-7-z[t]+jj]
    s0 = np.arange(T) - 7 - z
    return vf, s0


def _lpc1(e, a):
    x = np.empty_like(e)
    prev = 0.0
    for t in range(len(e)):
        prev = e[t] - a[t] * prev
        x[t] = prev
    return x


# ------------------------------------------------------------ blocked plan
def _m_split(t0, t1):
    """Split output range [t0,t1) into uniform M=32 pieces: the PE pays
    ~100ns to switch tile_size, but a homogeneous (128,32) stream with
    cycling col-quadrant t0 runs at ~20ns/piece."""
    return [(a, a + 32) for a in range(t0, t1, 32)]


def _plan_group(s0p, k, a, b):
    """For chunk-group [a*32, b*32) of round k, find the best window.
    Returns (cost, info) where info = (w0r, P, tc) with P the column
    boundary position (or None) and tc the straddle chunk start (or
    None), or None if infeasible."""
    base = k * B
    t0, t1 = a * 32, b * 32
    seg = s0p[base + t0: base + t1]
    lo = int(seg.min()) + OFFC * B
    hi = int(seg.max()) + 6 + OFFC * B
    if hi - lo + 1 > B:
        return None
    wlo = -(-(hi - (B - 1)) // 32)
    whi = lo // 32
    best = None
    for wq in range(whi, wlo - 1, -1):
        w0r = wq * 32
        if lo < w0r or hi >= w0r + B:
            continue
        P = (w0r // B + 1) * B         # column boundary position
        if P > hi:                     # whole band in col c1
            npieces = len(_m_split(t0, t1))
            cost = 30 * npieces
            cand = (cost, (w0r, None, None))
        else:
            # find chunk columns; s0 is nondecreasing in t
            tc = None
            ca_end = t0                # end of colA chunks (before straddle)
            cb_start = t1
            ok = True
            for c in range(a, b):
                clo = int(s0p[base + c * 32: base + (c + 1) * 32].min()) \
                    + OFFC * B
                chi = int(s0p[base + c * 32: base + (c + 1) * 32].max()) \
                    + 6 + OFFC * B
                if chi < P:
                    ca_end = (c + 1) * 32
                elif clo >= P:
                    cb_start = min(cb_start, c * 32)
                else:
                    if tc is not None:
                        ok = False
                        break
                    tc = c * 32
            if not ok:
                continue
            # A covers colA chunks + straddle; B covers colB chunks
            a_end = (tc + 32) if tc is not None else ca_end
            b_start = cb_start
            np_ = 0
            if a_end > t0:
                np_ += len(_m_split(t0, a_end))
            if t1 > b_start:
                np_ += len(_m_split(b_start, t1))
            ns = 1 if tc is not None else 0
            cost = 30 * (np_ + ns) + 30 * ns   # extra DMA penalty
            cand = (cost, (w0r, P, tc))
        if best is None or cand[0] < best[0]:
            best = cand
    return best


def _build_plan(vf, s0):
    """plan[k] = [('v'|'e', idx, col, t0, t1)] all full-K pieces;
    vtiles (NR,128,128) float64; etiles list of (128,32) float64."""
    s0p = np.concatenate([s0, s0[-1] + 1 + np.arange(TP - T)])
    vfp = np.concatenate([vf, np.zeros((TP - T, 7))]).astype(np.float64)
    vtiles = np.zeros((NR, B, B), np.float64)
    etiles = []
    ecross_round = []
    plan = []
    INF = 10 ** 9
    NC4 = 4
    for k in range(NR):
        base = k * B
        cost = {}
        for a in range(NC4):
            for b in range(a + 1, NC4 + 1):
                r = _plan_group(s0p, k, a, b)
                if r is not None:
                    cost[(a, b)] = r
        dp = [(INF, None)] * (NC4 + 1)
        dp[0] = (0, None)
        for b in range(1, NC4 + 1):
            for a in range(b):
                if (a, b) in cost and dp[a][0] + cost[(a, b)][0] < dp[b][0]:
                    dp[b] = (dp[a][0] + cost[(a, b)][0], a)
        assert dp[NC4][0] < INF, f"round {k}: no feasible plan"
        groups = []
        b = NC4
        while b > 0:
            a = dp[b][1]
            groups.append((a, b, cost[(a, b)][1]))
            b = a
        groups.reverse()

        pieces = []
        for (a, b, (w0r, P, tc)) in groups:
            t0, t1 = a * 32, b * 32
            c1 = w0r // B
            r0 = w0r % B
            col_a = c1
            # write taps & assign pieces
            if P is None:
                # all taps in col c1
                for tt in range(t0, t1):
                    tg = base + tt
                    for jj in range(7):
                        p = int(s0p[tg]) + jj + OFFC * B
                        vtiles[k, p % B, tt] += vfp[tg, jj]
                for (m0, m1) in _m_split(t0, t1):
                    pieces.append(('v', k, col_a, m0, m1))
            else:
                a_end = (tc + 32) if tc is not None else None
                et = None
                if tc is not None:
                    et = np.zeros((B, 32), np.float64)
                # chunk col assignment identical to _plan_group
                cb_start = t1
                ca_end = t0
                for c in range(a, b):
                    clo = int(s0p[base + c * 32: base + (c + 1) * 32].min())\
                        + OFFC * B
                    chi = int(s0p[base + c * 32: base + (c + 1) * 32].max())\
                        + 6 + OFFC * B
                    if chi < P:
                        ca_end = (c + 1) * 32
                    elif clo >= P:
                        cb_start = min(cb_start, c * 32)
                for tt in range(t0, t1):
                    tg = base + tt
                    in_straddle = tc is not None and tc <= tt < tc + 32
                    for jj in range(7):
                        p = int(s0p[tg]) + jj + OFFC * B
                        if in_straddle and p >= P:
                            et[p % B, tt - tc] += vfp[tg, jj]
                        else:
                            vtiles[k, p % B, tt] += vfp[tg, jj]
                a_end2 = (tc + 32) if tc is not None else ca_end
                if a_end2 > t0:
                    for (m0, m1) in _m_split(t0, a_end2):
                        pieces.append(('v', k, col_a, m0, m1))
                if t1 > cb_start:
                    for (m0, m1) in _m_split(cb_start, t1):
                        pieces.append(('v', k, col_a + 1, m0, m1))
                if tc is not None:
                    ei = len(etiles)
                    etiles.append(et)
                    ecross_round.append(k)
                    pieces.append(('e', ei, col_a + 1, tc, tc + 32))
        plan.append(pieces)
    return plan, vtiles, etiles, ecross_round


def _group_bounds():
    gb = [0, 2, 4, 8]
    while gb[-1] < NR:
        gb.append(min(gb[-1] + GRP, NR))
    g_of = np.zeros(NR, np.int64)
    for g in range(len(gb) - 1):
        g_of[gb[g]:gb[g + 1]] = g
    return gb, g_of


def _sub_iter(m):
    """Iteration at which pair m's sub (rounds 2m, 2m+1) is emitted."""
    return min(2 * m + 1, NR - 1)


def _schedule(plan):
    """sched[k] = [(src, idx, col, t0, t1, start, stop)] emitted at
    iteration k, readiness-ordered: piece reading col c is ready after
    the PAIR sub of pair (c-OFFC)//2; emit at e = clamp(prod+1,
    group_start, j). start/stop per (round, t-range) region in emission
    order."""
    gb, g_of = _group_bounds()
    items = []
    for j, pieces in enumerate(plan):
        for (src, idx, col, t0, t1) in pieces:
            d = col - OFFC
            e = max(d + 2, gb[int(g_of[j])])
            e = min(e, j)
            items.append((e, d, j != e, j, src == 'e',
                          (src, idx, col, t0, t1)))
    items.sort(key=lambda it: (it[0], it[1], it[2], it[3], it[4]))
    first_of, last_of = {}, {}
    for i, (e, d, fut, j, isex, p) in enumerate(items):
        # region key: overlapping ranges share a region via t0 anchor:
        # 'e' pieces overlap the colA piece containing their range, so
        # anchor regions by (j, covering piece span). Use (j, t0, t1) and
        # mark 'e' pieces as accumulating into their covering region.
        tr = (j, p[3], p[4])
        if tr not in first_of:
            first_of[tr] = i
        last_of[tr] = i
    sched = [[] for _ in range(NR)]
    for i, (e, d, fut, j, isex, p) in enumerate(items):
        (src, idx, col, t0, t1) = p
        if src == 'e':
            st, sp = False, True
        else:
            tr = (j, t0, t1)
            st, sp = first_of[tr] == i, last_of[tr] == i
        sched[e].append((src, idx, j, col, t0, t1, st, sp))
    return sched


# ------------------------------------------------------------- device build
def _build_kernel(plan, n_etiles, ecross_round):
    sched = _schedule(plan)
    gb, g_of = _group_bounds()
    NX = max(1, n_etiles)
    NEC = (NX + ECHUNK - 1) // ECHUNK

    nc = bacc.Bacc("TRN2", target_bir_lowering=False, debug=False)
    v_d = nc.dram_tensor("vtiles", [B, NR, B], FP16, kind="ExternalInput")
    e_d = nc.dram_tensor("etiles", [B, NEC * ECHUNK, 32], FP16,
                         kind="ExternalInput")
    x_d = nc.dram_tensor("xcols", [B, NR + 1], F32, kind="ExternalInput")
    id_d = nc.dram_tensor("ident", [B, B], FP16, kind="ExternalInput")
    y_d = nc.dram_tensor("y", [SLOTS * NPH * B], F32, kind="ExternalOutput")

    ident_act = mybir.ActivationFunctionType.Identity
    act_copy = mybir.ActivationFunctionType.Copy

    # iteration at which to issue each extra chunk DMA
    echunk_issue = {}
    for c in range(NEC):
        i0 = c * ECHUNK
        rnd = ecross_round[i0] if i0 < len(ecross_round) else 0
        echunk_issue.setdefault(max(0, rnd - 30), []).append(c)

    with TileContext(nc) as tc:
        with (
            tc.tile_pool(name="vpool", bufs=4) as vpool,
            tc.tile_pool(name="epool", bufs=1) as epool,
            tc.tile_pool(name="hpool", bufs=1) as hpool,
            tc.tile_pool(name="xpool", bufs=1) as xpool,
            tc.tile_pool(name="ps", bufs=NBANK, space="PSUM") as ps,
            tc.tile_pool(name="pso", bufs=2, space="PSUM") as pso,
            tc.tile_pool(name="opool", bufs=2) as opool,
        ):
            h_ph = []
            for i in range(NPH):
                ht = hpool.tile([B, SLOTS], FP16, tag=f"h{i}", name=f"h{i}")
                nc.vector.memset(ht[:, :], 0.0)
                h_ph.append(ht)
            hz = hpool.tile([B, OFFC], FP16, tag="hz", name="hz")
            nc.vector.memset(hz[:, :], 0.0)

            def hcol_ap(c):
                if c < OFFC:
                    return hz[:, c:c + 1]
                cp, par = (c - OFFC) // 2, (c - OFFC) % 2
                s = 2 * (cp // NPH) + par
                return h_ph[cp % NPH][:, s:s + 1]

            xt = xpool.tile([B, NR + 1], F32)
            nc.sync.dma_start(xt[:, :], x_d[:, :])
            idt = xpool.tile([B, B], FP16, tag="ident")
            nc.sync.dma_start(idt[:, :], id_d[:, :])
            et_all = epool.tile([B, NEC * ECHUNK, 32], FP16, tag="ex")

            vtiles_sb = {}
            accs = {}
            for k in range(NR):
                g = int(g_of[k])
                if k == gb[g]:
                    gn = gb[g + 1] - gb[g]
                    vt = vpool.tile([B, GRP, B], FP16, tag="v", name=f"v{g}")
                    eng = nc.sync if (g % 2 == 0) else nc.gpsimd
                    eng.dma_start(vt[:, 0:gn, :], v_d[:, gb[g]:gb[g + 1], :])
                    vtiles_sb[g] = vt
                for c in echunk_issue.get(k, []):
                    nc.gpsimd.dma_start(
                        et_all[:, c * ECHUNK:(c + 1) * ECHUNK, :],
                        e_d[:, c * ECHUNK:(c + 1) * ECHUNK, :])
                for (src, idx, j, col, t0, t1, st, sp) in sched[k]:
                    if j not in accs:
                        accs[j] = ps.tile([B, 1], F32, tag="acc",
                                          name=f"acc{j}")
                    if src == 'v':
                        vt = vtiles_sb[int(g_of[j])]
                        kk = j - gb[int(g_of[j])]
                        lhsT = vt[:, kk, t0:t1]
                    else:
                        lhsT = et_all[:, idx, 0:t1 - t0]
                    nc.tensor.matmul(
                        accs[j][t0:t1, :], lhsT, hcol_ap(col),
                        start=st, stop=sp, tile_position=(0, t0),
                        skip_group_check=True)
                # h_col = x - acc (fused, fp16 out), alternating DVE/ACT
                acc = accs.pop(k)
                if k % 2 == 0:
                    nc.vector.tensor_sub(hcol_ap(k + OFFC), xt[:, k:k + 1],
                                         acc[:, :])
                else:
                    nc.scalar.activation(hcol_ap(k + OFFC), acc[:, :],
                                         ident_act, bias=xt[:, k:k + 1],
                                         scale=-1.0)

            # ---- output: per-phase transpose of history -> linear y
            # phase i slot 2q+par = pair cp = 8q+i, round k = 2cp+par =
            # 16q + 2i + par; y viewed [q, w=16, p]: w = 2i + par.
            y4 = y_d.rearrange("(q w p) -> q w p", w=2 * NPH, p=B)
            S2 = SLOTS // 2
            for i in range(NPH):
                for par in (0, 1):
                    tp = pso.tile([S2, B], FP16, tag="tp", name=f"tp{i}{par}")
                    nc.tensor.transpose(tp[:, :], h_ph[i][:, par:SLOTS:2],
                                        idt[:, :])
                    osb = opool.tile([S2, B], F32, tag="o", name=f"o{i}{par}")
                    if par == 0:
                        nc.vector.tensor_copy(osb[:, :], tp[:, :])
                    else:
                        nc.scalar.activation(osb[:, :], tp[:, :], act_copy)
                    nc.sync.dma_start(y4[0:S2, 2 * i + par, :], osb[:, :])
    nc.compile()
    return nc


# --------------------------------------------------------------- entry point
_CACHE = {}


def kernel(delay_len_frames, raw_gain, raw_coeff_frames, excitation,
           exc_coefficients, n_samples):
    delay_len_frames = np.asarray(delay_len_frames, np.float32)
    raw_gain = np.asarray(raw_gain, np.float32)
    raw_coeff_frames = np.asarray(raw_coeff_frames, np.float32)
    excitation = np.asarray(excitation, np.float32)
    exc_coefficients = np.asarray(exc_coefficients, np.float32)
    assert int(n_samples) == T

    vf, s0 = _host_structure(delay_len_frames, raw_gain[0], raw_coeff_frames)
    plan, vtiles, etiles, ecross_round = _build_plan(vf, s0)

    vpack = np.ascontiguousarray(
        vtiles.astype(np.float16).transpose(1, 0, 2))
    NX = max(1, len(etiles))
    NEC = (NX + ECHUNK - 1) // ECHUNK
    epack = np.zeros((B, NEC * ECHUNK, 32), np.float16)
    for i, et in enumerate(etiles):
        epack[:, i, :] = et.astype(np.float16)

    x = _lpc1(np.float64(excitation), np.float64(exc_coefficients[0, :, 0]))
    xp = np.zeros((NR + 1) * B, np.float32)
    xp[:T] = x.astype(np.float32)
    xcols = np.ascontiguousarray(xp.reshape(NR + 1, B).T)   # [128, NR+1]

    key = hash((delay_len_frames.tobytes(), raw_gain.tobytes(),
                raw_coeff_frames.tobytes()))
    if key not in _CACHE:
        _CACHE[key] = _build_kernel(plan, len(etiles), ecross_round)
    nc = _CACHE[key]

    in_map = dict(vtiles=vpack, etiles=epack, xcols=xcols,
                  ident=np.eye(B, dtype=np.float16))
    res = run_bass_kernel_spmd(nc, [in_map], core_ids=[0], trace=TRACE)
    if TRACE:
        global LAST_EXEC_NS, LAST_RES
        LAST_EXEC_NS = res.exec_time_ns
        LAST_RES = res
    y = res.results[0]["y"]
    return np.asarray(y[:T], np.float32)


if __name__ == "__main__":
    rng = np.random.default_rng(0)
    out = kernel(
        delay_len_frames=300 + 200 * rng.random(NFRAMES).astype(np.float32),
        raw_gain=np.full(1, 2.5, np.float32),
        raw_coeff_frames=(-2 * rng.random((NFRAMES, NCOEF))).astype(np.float32),
        excitation=rng.standard_normal(T).astype(np.float32),
        exc_coefficients=0.01 * rng.standard_normal((1, T, 1)).astype(np.float32),
        n_samples=T)
    print("kernel ran, out:", out.shape, out[:4])
